# revision 7
# baseline (speedup 1.0000x reference)
"""GCGRU (Chebyshev graph-conv GRU) Trainium2 Bass kernel.

Full-input contract: kernel(**inputs) takes the complete tensors and returns
the complete [16, 2048, 64] output. Internally shards batch across 8 cores
(2 batches per core); the adjacency work is replicated per core (it is cheap
relative to moving it between cores).

Math (per core, b = 2 local batches, N = 2048 nodes):
  U = exp(relu(E @ E.T))            # symmetric! stored bf16 in SBUF
  rs = row-sums of U; A = diag(1/rs) @ U
  X = concat(x, state)              # [N, b, 66] layout: node on partitions
  T0 = X; T1 = A @ X; T2 = 2 A @ T1 - T0   (Chebyshev on products, no A^2)
  z_r = sigmoid(sum_d E[:, d] * (x_g @ pool_g[d]) + bias_g)
  cand = concat(x, r * state); same pipeline with update pool -> h
  out = z * state + (1 - z) * h

The per-node weights W[n] = sum_d E[n,d] pool[d] are never materialized:
the projection runs per Chebyshev-feature against each pool[d] slice and the
d-contraction is fused scalar_tensor_tensor MACs with per-partition E scalars.
"""

import os
import sys
import time

import numpy as np

for _p in ("/opt/trn_rl_repo", "/root/.axon_site/_ro/trn_rl_repo"):
    if os.path.isdir(_p) and _p not in sys.path:
        sys.path.insert(0, _p)

import concourse.bass as bass
import concourse.mybir as mybir
import concourse.tile as tile
from concourse import bacc
from concourse.bass_utils import run_bass_kernel_spmd
from concourse.masks import make_identity

F32 = mybir.dt.float32
F32R = mybir.dt.float32r
BF16 = mybir.dt.bfloat16
AF = mybir.ActivationFunctionType
ALU = mybir.AluOpType

# Problem shape (hardcoded per contract)
B, N, CI, H, D, K = 16, 2048, 2, 64, 10, 3
NCORES = 8
BL = B // NCORES          # batches per core = 2
C = CI + H                # 66
NT = N // 128             # 16 node tiles
OG = 2 * H                # 128 gate output
OU = H                    # 64 update output


def _build_gcn(nc, tc, pools, bufs, which):
    """One cheb_gcn + pointwise epilogue. which in ('gate', 'update')."""
    (persist, psum_a, psum_tr, psum_y, work) = pools
    (U_sb, Xf, Xb, rs_inv, rs2_inv, E_sb, ET, id_bf, zr_sb, X2b,
     pg, gb_sb, pu, ub_sb, out_sb) = bufs

    if which == "gate":
        rhs0 = Xb
        ps, bias_sb, OD = pg, gb_sb, OG
    else:
        rhs0 = X2b
        ps, bias_sb, OD = pu, ub_sb, OU

    # ---- graph conv: T1 = A @ X, T2 = 2 A @ T1 - X --------------------
    # lhsT tile for (m-chunk j, n-tile i) is U[j*128: , i*128: ] which by
    # symmetry equals U[n, m] as required.
    T1 = persist.tile([128, NT, BL, C], BF16, tag=f"T1{which}")
    T2 = persist.tile([128, NT, BL, C], BF16, tag=f"T2{which}")
    for i in range(NT):
        acc = psum_a.tile([128, BL * C], F32, tag="acc")
        for j in range(NT):
            nc.tensor.matmul(
                acc[:],
                U_sb[:, j, i * 128:(i + 1) * 128],
                rhs0[:, j, :, :],
                start=(j == 0), stop=(j == NT - 1),
            )
        # T1 = rs_inv * raw   (softmax normalization deferred to here)
        nc.vector.tensor_scalar_mul(
            T1[:, i, :, :], acc[:].rearrange("p (b c) -> p b c", b=BL),
            rs_inv[:, i, :])
    for i in range(NT):
        acc = psum_a.tile([128, BL * C], F32, tag="acc")
        for j in range(NT):
            nc.tensor.matmul(
                acc[:],
                U_sb[:, j, i * 128:(i + 1) * 128],
                T1[:, j, :, :],
                start=(j == 0), stop=(j == NT - 1),
            )
        # T2 = 2 * rs_inv * raw - T0
        nc.vector.scalar_tensor_tensor(
            T2[:, i, :, :], acc[:].rearrange("p (b c) -> p b c", b=BL),
            rs2_inv[:, i, :], rhs0[:, i, :, :],
            op0=ALU.mult, op1=ALU.subtract)

    # ---- projection + epilogue per (n-tile, local batch) row tile -----
    for t in range(NT):
        for b in range(BL):
            # transpose x_g = [T0 | T1 | T2] slices into [c, rows] layout,
            # one 66-partition K-chunk per Chebyshev order
            xgTs = []
            for k, src in enumerate((rhs0, T1, T2)):
                xg = work.tile([66, 128], BF16, tag=f"xgT{k}")
                ptr = psum_tr.tile([66, 128], BF16, tag="tr0")
                nc.tensor.transpose(ptr[:], src[:, t, b, :], id_bf[:])
                nc.scalar.activation(xg[:], ptr[:], AF.Copy)
                xgTs.append(xg[:])

            # Y[rows, d, o] = sum_c xg[c, rows] pool[c, d, o], in 2 d-halves
            tmps = []
            for half in range(2):
                py = psum_y.tile([128, 5, OD], F32, tag="y")
                dlo = half * 5
                fchunks = [(0, 5)] if OD == OU else [(0, 4), (4, 5)]
                for (f0, f1) in fchunks:
                    for k in range(3):
                        nc.tensor.matmul(
                            py[:, f0:f1, :], xgTs[k],
                            ps[k][:, dlo + f0:dlo + f1, :],
                            start=(k == 0), stop=(k == 2))
                # d-contraction: s += E[:, d] * Y_d  (fused MACs)
                ta = work.tile([128, OD], F32, tag="dca")
                tb = work.tile([128, OD], F32, tag="dcb")
                src = bias_sb[:, t, :] if half == 0 else tmps[0]
                for d in range(5):
                    dst = ta if d % 2 == 0 else tb
                    nc.vector.scalar_tensor_tensor(
                        dst[:], py[:, d, :], E_sb[:, t, dlo + d:dlo + d + 1],
                        src[:], op0=ALU.mult, op1=ALU.add)
                    src = dst
                tmps.append(src)
            pre = tmps[1]

            if which == "gate":
                nc.scalar.activation(zr_sb[:, t, b, :], pre[:], AF.Sigmoid)
                # cand state part: X2[:, 2:66] = r * state
                nc.vector.tensor_tensor(
                    X2b[:, t, b, CI:C], zr_sb[:, t, b, H:OG],
                    Xf[:, t, b, CI:C], op=ALU.mult)
            else:
                hh = work.tile([128, OU], F32, tag="hh")
                nc.scalar.activation(hh[:], pre[:], AF.Tanh)
                # out = z*state + (1-z)*h = h + z*(state - h)
                vv = work.tile([128, OU], F32, tag="vv")
                nc.vector.tensor_tensor(
                    vv[:], Xf[:, t, b, CI:C], hh[:], op=ALU.subtract)
                nc.vector.tensor_tensor(
                    vv[:], zr_sb[:, t, b, 0:H], vv[:], op=ALU.mult)
                nc.vector.tensor_tensor(
                    out_sb[:, t, b, :], vv[:], hh[:], op=ALU.add)


def build_program():
    nc = bacc.Bacc("TRN2", target_bir_lowering=False, debug=False,
                   enable_asserts=False, num_devices=NCORES)

    x_d = nc.dram_tensor("x", [BL, N, CI], F32, kind="ExternalInput").ap()
    st_d = nc.dram_tensor("state", [BL, N, H], F32, kind="ExternalInput").ap()
    e_d = nc.dram_tensor("node_embeddings", [N, D], F32,
                         kind="ExternalInput").ap()
    gw_d = nc.dram_tensor("gate_weights_pool", [D, K, C, OG], F32,
                          kind="ExternalInput").ap()
    gb_d = nc.dram_tensor("gate_bias_pool", [D, OG], F32,
                          kind="ExternalInput").ap()
    uw_d = nc.dram_tensor("update_weights_pool", [D, K, C, OU], F32,
                          kind="ExternalInput").ap()
    ub_d = nc.dram_tensor("update_bias_pool", [D, OU], F32,
                          kind="ExternalInput").ap()
    out_d = nc.dram_tensor("out", [BL, N, H], F32, kind="ExternalOutput").ap()

    with tile.TileContext(nc) as tc:
        import contextlib
        with contextlib.ExitStack() as ctx:
            persist = ctx.enter_context(tc.tile_pool(name="persist", bufs=1))
            work = ctx.enter_context(tc.tile_pool(name="work", bufs=3))
            psum_a = ctx.enter_context(
                tc.tile_pool(name="psum_a", bufs=2, space="PSUM"))
            psum_tr = ctx.enter_context(
                tc.tile_pool(name="psum_tr", bufs=2, space="PSUM"))
            psum_y = ctx.enter_context(
                tc.tile_pool(name="psum_y", bufs=2, space="PSUM"))
            pools = (persist, psum_a, psum_tr, psum_y, work)

            # ---- constants / inputs into SBUF -------------------------
            id_f32 = persist.tile([128, 128], F32, tag="id_f32")
            make_identity(nc, id_f32[:])
            id_bf = persist.tile([128, 128], BF16, tag="id_bf")
            make_identity(nc, id_bf[:])

            E_sb = persist.tile([128, NT, D], F32, tag="E_sb")
            nc.sync.dma_start(E_sb[:], e_d.rearrange("(t p) d -> p t d", p=128))

            gwr = gw_d.rearrange("d k i o -> (k i) d o")
            uwr = uw_d.rearrange("d k i o -> (k i) d o")
            pg, pu = [], []
            for k in range(K):
                stg = work.tile([C, D, OG], F32, tag="pool_stage")
                nc.sync.dma_start(stg[:], gwr[k * C:(k + 1) * C])
                t_g = persist.tile([C, D, OG], BF16, tag=f"pg{k}")
                nc.vector.tensor_copy(t_g[:], stg[:])
                pg.append(t_g)
                stg = work.tile([C, D, OG], F32, tag="pool_stage")
                nc.sync.dma_start(stg[:, :, 0:OU], uwr[k * C:(k + 1) * C])
                t_u = persist.tile([C, D, OU], BF16, tag=f"pu{k}")
                nc.vector.tensor_copy(t_u[:], stg[:, :, 0:OU])
                pu.append(t_u)
            gbp = persist.tile([D, OG], F32, tag="gbp")
            nc.sync.dma_start(gbp[:], gb_d[:])
            ubp = persist.tile([D, OU], F32, tag="ubp")
            nc.sync.dma_start(ubp[:], ub_d[:])

            Xf = persist.tile([128, NT, BL, C], F32, tag="Xf")
            Xb = persist.tile([128, NT, BL, C], BF16, tag="Xb")
            for b in range(BL):
                nc.sync.dma_start(
                    Xf[:, :, b, 0:CI],
                    x_d[b].rearrange("(t p) c -> p t c", p=128))
                nc.sync.dma_start(
                    Xf[:, :, b, CI:C],
                    st_d[b].rearrange("(t p) c -> p t c", p=128))
            nc.vector.tensor_copy(Xb[:], Xf[:])

            # ---- E^T via PE transposes --------------------------------
            ET = persist.tile([D, N], F32, tag="ET")
            for t in range(NT):
                pt = psum_tr.tile([D, 128], F32, tag="tr0")
                nc.tensor.transpose(pt[:], E_sb[:, t, :], id_f32[:])
                nc.scalar.activation(ET[:, t * 128:(t + 1) * 128], pt[:],
                                     AF.Copy)

            # ---- U = exp(relu(E E^T)), rowsums ------------------------
            U_sb = persist.tile([128, NT, N], BF16, tag="U_sb")
            for i in range(NT):
                for jc in range(N // 512):
                    pz = psum_a.tile([128, 512], F32, tag="acc")
                    nc.tensor.matmul(
                        pz[:], ET[:, i * 128:(i + 1) * 128],
                        ET[:, jc * 512:(jc + 1) * 512],
                        start=True, stop=True)
                    nc.vector.tensor_scalar_max(pz[:], pz[:], 0.0)
                    nc.scalar.activation(
                        U_sb[:, i, jc * 512:(jc + 1) * 512], pz[:], AF.Exp)
            rs = persist.tile([128, NT, 1], F32, tag="rs")
            nc.vector.tensor_reduce(rs[:], U_sb[:],
                                    axis=mybir.AxisListType.X, op=ALU.add)
            rs_inv = persist.tile([128, NT, 1], F32, tag="rs_inv")
            nc.vector.reciprocal(rs_inv[:], rs[:])
            rs2_inv = persist.tile([128, NT, 1], F32, tag="rs2_inv")
            nc.vector.tensor_scalar_mul(rs2_inv[:], rs_inv[:], 2.0)

            # ---- per-node biases: E @ bias_pool -----------------------
            gb_sb = persist.tile([128, NT, OG], F32, tag="gb_sb")
            ub_sb = persist.tile([128, NT, OU], F32, tag="ub_sb")
            for t in range(NT):
                pb = psum_tr.tile([128, OG], F32, tag="tr0")
                nc.tensor.matmul(pb[:], ET[:, t * 128:(t + 1) * 128], gbp[:],
                                 start=True, stop=True)
                nc.scalar.activation(gb_sb[:, t, :], pb[:], AF.Copy)
                pb = psum_tr.tile([128, OG], F32, tag="tr0")
                nc.tensor.matmul(pb[:, 0:OU], ET[:, t * 128:(t + 1) * 128],
                                 ubp[:], start=True, stop=True)
                nc.scalar.activation(ub_sb[:, t, :], pb[:, 0:OU], AF.Copy)

            # ---- shared buffers for the two GCN passes ----------------
            zr_sb = persist.tile([128, NT, BL, OG], F32, tag="zr_sb")
            X2b = persist.tile([128, NT, BL, C], BF16, tag="X2b")
            # x columns of cand = x columns of X
            nc.vector.tensor_copy(X2b[:, :, :, 0:CI], Xf[:, :, :, 0:CI])
            out_sb = persist.tile([128, NT, BL, H], F32, tag="out_sb")

            bufs = (U_sb, Xf, Xb, rs_inv, rs2_inv, E_sb, ET, id_bf, zr_sb,
                    X2b, pg, gb_sb, pu, ub_sb, out_sb)
            _build_gcn(nc, tc, pools, bufs, "gate")
            _build_gcn(nc, tc, pools, bufs, "update")

            for b in range(BL):
                nc.sync.dma_start(
                    out_d[b].rearrange("(t p) o -> p t o", p=128),
                    out_sb[:, :, b, :])

    nc.compile()
    return nc


_NC_CACHE = {}
LAST_EXEC_NS = None
LAST_WALL_S = None


def kernel(x, state, node_embeddings, gate_weights_pool, gate_bias_pool,
           update_weights_pool, update_bias_pool):
    global LAST_EXEC_NS, LAST_WALL_S
    if "nc" not in _NC_CACHE:
        _NC_CACHE["nc"] = build_program()
    nc = _NC_CACHE["nc"]

    x = np.ascontiguousarray(np.asarray(x, dtype=np.float32))
    state = np.ascontiguousarray(np.asarray(state, dtype=np.float32))
    shared = {
        "node_embeddings": np.ascontiguousarray(
            np.asarray(node_embeddings, np.float32)),
        "gate_weights_pool": np.ascontiguousarray(
            np.asarray(gate_weights_pool, np.float32)),
        "gate_bias_pool": np.ascontiguousarray(
            np.asarray(gate_bias_pool, np.float32)),
        "update_weights_pool": np.ascontiguousarray(
            np.asarray(update_weights_pool, np.float32)),
        "update_bias_pool": np.ascontiguousarray(
            np.asarray(update_bias_pool, np.float32)),
    }
    in_maps = []
    for c in range(NCORES):
        m = dict(shared)
        m["x"] = x[c * BL:(c + 1) * BL]
        m["state"] = state[c * BL:(c + 1) * BL]
        in_maps.append(m)

    trace = bool(int(os.environ.get("KERNEL_TRACE", "0")))
    t0 = time.perf_counter()
    res = run_bass_kernel_spmd(nc, in_maps, list(range(NCORES)), trace=trace)
    LAST_WALL_S = time.perf_counter() - t0
    LAST_EXEC_NS = res.exec_time_ns

    out = np.concatenate([res.results[c]["out"] for c in range(NCORES)],
                         axis=0)
    return out.astype(np.float32)
